# revision 20
# baseline (speedup 1.0000x reference)
"""Trainium2 Bass kernel for nn_NodeModel (GNN message passing).

  out = relu(concat([x, scatter_mean(edge_attr, col), u[batch]]) @ W1 + b1) @ W2 + b2

v6 = v4 restructured around measured per-instruction costs (HW matmul
instr ~195ns regardless of dtype/DoubleRow; DVE op ~290ns; Act relu
~570ns; DMA ~344GB/s/queue):

  * One merged [128, C] bf16 matmul replaces the separate x and
    u-one-hot passes: rhs rows 0:64 = x^T (DMA bf16), rows 64:128 =
    one-hot(batch) built on device from a [1, SLOTS] graph-id row
    (26KB) via gpsimd partition_broadcast + one DVE is_equal per
    chunk.  lhsT = [W1x; hu].  -25 PE instrs, -0.8MB HBM.
  * W2 matmul of group k is deferred two groups so the PE never waits
    on the Act engine's relu latency; hid/psh/pso buffers deepened to
    match (PE p-state ramp needs continuous busy).
  * Edge chunks alternate between the SP and Act HWDGE queues; xoh
    prefetch is issued one chunk ahead on the gpsimd queue, with the
    (data-dependent) out DMA behind it so prefetch never stalls.
  * No cross-core communication: edges live with their destination node.
"""

import numpy as np

try:
    import ml_dtypes

    _BF16 = np.dtype(ml_dtypes.bfloat16)
    _FP8 = np.dtype(ml_dtypes.float8_e4m3fn)
except Exception:  # pragma: no cover
    _BF16 = None
    _FP8 = None

F_E, F_X, F_U, H, F_OUT = 16, 64, 64, 128, 64

CFG = dict(
    n_cores=8,
    n_nodes=100000,
    n_graphs=64,
    ng=25,        # groups per core
    cols=512,     # nodes per group (matmul moving dim)
    out_batch=5,  # groups per output DMA
    in_batch=5,   # groups per x/one-hot DMA
    et_chunk=5,   # groups per edge DMA
    dev_oh="bcast",  # one-hot: partition_broadcast(gid) + DVE is_equal
    et_split=True,   # alternate edge chunks across SP/Act HWDGE queues
    edge_bufs=4,
    defer=2,      # groups of W2 deferral (hides relu latency from PE)
    hid_bufs=4,
    psh_bufs=4,
    pso_bufs=4,
)

_CACHE = {}


# ---------------------------------------------------------------- host side
def _plan(col, cfg):
    """Degree-sorted node permutation and per-group-slot plane schedule."""
    NC, NG, COLS = cfg["n_cores"], cfg["ng"], cfg["cols"]
    NPAD = NC * NG * COLS
    cnt = np.bincount(col, minlength=NPAD)  # pad nodes have degree 0
    order = np.argsort(cnt, kind="stable").astype(np.int64)  # ascending degree
    deg_sorted = cnt[order]
    gmax = deg_sorted.reshape(NC * NG, COLS).max(1)
    nps = np.ceil(gmax.reshape(NG, NC).max(1) / 8.0).astype(np.int64)
    nps = np.maximum(nps, 1)  # planes of 8 edge slots per group
    gi = np.arange(NPAD, dtype=np.int32) // COLS
    core = np.empty(NPAD, np.int32)
    kslot = np.empty(NPAD, np.int32)
    colidx = np.empty(NPAD, np.int32)
    core[order] = gi % NC
    kslot[order] = gi // NC
    colidx[order] = np.arange(NPAD, dtype=np.int32) % COLS
    # node_at[c, slot]: node id occupying (core c, slot k*COLS+ci)
    node_at = np.empty(NPAD, np.int64)
    pos = (gi % NC).astype(np.int64) * (NG * COLS) \
        + (gi // NC).astype(np.int64) * COLS \
        + np.arange(NPAD, dtype=np.int64) % COLS
    node_at[pos] = order
    node_at = node_at.reshape(NC, NG * COLS)
    return cnt, core, kslot, colidx, node_at, tuple(int(v) for v in nps)


def _preprocess(inputs, cfg):
    NC, NG, COLS = cfg["n_cores"], cfg["ng"], cfg["cols"]
    N, GR = cfg["n_nodes"], cfg["n_graphs"]
    SLOTS = NG * COLS

    x = np.asarray(inputs["x"], np.float32)
    ea = np.asarray(inputs["edge_attr"], np.float32)
    u = np.asarray(inputs["u"], np.float32)
    W1 = np.asarray(inputs["W1"], np.float32)
    b1 = np.asarray(inputs["b1"], np.float32)
    W2 = np.asarray(inputs["W2"], np.float32)
    b2 = np.asarray(inputs["b2"], np.float32)
    col = np.asarray(np.asarray(inputs["edge_index"])[1], np.int64)
    batch = np.asarray(inputs["batch"], np.int64)
    assert x.shape[0] == N and u.shape[0] == GR

    cnt, core, kslot, colidx, node_at, nps = _plan(col, cfg)
    cfg["nps"] = nps
    NPS = np.array(nps, np.int32)
    off = ((np.cumsum(NPS) - NPS) * COLS).astype(np.int32)  # plane offsets
    TOT = int(NPS.sum()) * COLS

    invc = np.zeros(cnt.shape[0], np.float32)
    nz = cnt > 0
    invc[nz] = 1.0 / cnt[nz]

    order = np.argsort(col, kind="stable")
    cols_s = col[order]
    eas = np.clip(ea[order] * invc[cols_s][:, None], -240.0, 240.0)
    eq = eas.astype(_FP8).view(np.uint8)  # [E, 16]

    starts = (np.cumsum(cnt) - cnt).astype(np.int64)
    rank = (np.arange(col.shape[0], dtype=np.int64) - starts[cols_s]).astype(
        np.int32)
    c = core[cols_s]
    k = kslot[cols_s]
    ci = colidx[cols_s]
    pl = rank >> 3        # plane within group
    s = rank & 7          # slot within plane

    # edges[c][part = s*16+f][off_k + pl*COLS + ci]  (fp8)
    A = np.zeros((NC, 128, TOT), np.uint8)
    free = off[k] + pl * COLS + ci
    base = (c * 128 + (s << 4)).astype(np.int64) * TOT + free
    fidx = (np.arange(F_E, dtype=np.int64) * TOT)[None, :]
    A.reshape(-1)[base[:, None] + fidx] = eq
    A = A.view(_FP8)

    # x features transposed into permuted slots (bf16), one-hot graph id (fp8)
    slot = (kslot[:N].astype(np.int64) * COLS + colidx[:N])
    xq = np.zeros((N + 1, F_X), _BF16)
    xq[:N] = x.astype(_BF16)
    nat = np.minimum(node_at, N)  # pad nodes -> zero row N
    xT = np.ascontiguousarray(
        xq[nat].transpose(0, 2, 1))  # [NC, F_X, SLOTS]
    bq = np.full(N + 1, GR, np.int32)
    bq[:N] = batch.astype(np.int32)
    oh_mode = cfg.get("dev_oh", False)
    if oh_mode:  # True (PE synth) or "bcast" (partition_broadcast + is_equal)
        # graph-id per slot (bf16 holds ints <= 256 exactly); pads get GR,
        # which matches no iota row -> all-zero one-hot column
        gid = bq[nat].astype(np.float32).astype(_BF16)[:, None, :]  # [NC,1,SLOTS]
        oh = None
    else:
        one = np.float32(1.0).astype(_FP8).view(np.uint8).item()
        ohw = np.zeros((NC, GR + 1, SLOTS), np.uint8)
        sl = np.arange(SLOTS)
        for cidx in range(NC):
            ohw[cidx, bq[nat[cidx]], sl] = one
        oh = np.ascontiguousarray(ohw[:, :GR]).view(_FP8)
        gid = None

    # W1 rows: x 0:64, e 64:80, u 80:144
    hu = u @ W1[F_X + F_E:]                                     # [64, H]
    W1xu = np.ascontiguousarray(
        np.concatenate([W1[0:F_X], hu], axis=0), dtype=_BF16)   # [128, H]
    W1e = W1[F_X:F_X + F_E]                                     # [16, H]
    W1e8 = np.tile(np.clip(W1e, -240, 240), (8, 1))             # [128, H]
    W1e8d = np.ascontiguousarray(
        np.concatenate([W1e8, W1e8], axis=1), dtype=_FP8)       # [128, 2H] DR
    W1e8s = np.ascontiguousarray(W1e8, dtype=_FP8)              # [128, H] plain
    W2c = np.ascontiguousarray(W2, dtype=_BF16)                 # [128, 64]

    common = dict(
        w1xu=W1xu, w1e8d=W1e8d, w1e8s=W1e8s, w2=W2c,
        b1=np.ascontiguousarray(b1.reshape(H, 1), np.float32),
    )
    if oh_mode:
        common["iota"] = np.arange(GR, dtype=np.float32).reshape(GR, 1)
        if oh_mode is True:
            common["ones"] = np.ones((1, GR), _BF16)
    in_maps = []
    for cidx in range(NC):
        im = dict(common)
        im["edges"] = np.ascontiguousarray(A[cidx])
        im["xt"] = np.ascontiguousarray(xT[cidx])
        if oh_mode:
            im["gid"] = np.ascontiguousarray(gid[cidx])
        else:
            im["oh"] = np.ascontiguousarray(oh[cidx])
        in_maps.append(im)
    meta = dict(core=core[:N], slot=slot, b2=b2)
    return in_maps, meta


def _postprocess(results, meta, cfg):
    NC, NG, COLS = cfg["n_cores"], cfg["ng"], cfg["cols"]
    SLOTS = NG * COLS
    stack = np.stack(
        [np.asarray(results[c]["outT"]).astype(np.float32) for c in range(NC)]
    )  # [NC, F_OUT, NG, COLS]
    stack = stack.reshape(NC, F_OUT, SLOTS)
    out = stack[meta["core"], :, meta["slot"]]  # [N, 64]
    out += meta["b2"][None, :]
    return out


# ------------------------------------------------------------- device side
def _build(cfg):
    import concourse.bacc as bacc
    import concourse.mybir as mybir
    import concourse.tile as tile
    from contextlib import ExitStack

    NG, COLS, GR = cfg["ng"], cfg["cols"], cfg["n_graphs"]
    NPS = list(cfg["nps"])
    assert len(NPS) == NG
    SLOTS = NG * COLS
    TOT = int(sum(NPS)) * COLS
    off = np.concatenate([[0], np.cumsum(NPS)[:-1]]) * COLS
    f32 = mybir.dt.float32
    bf16 = mybir.dt.bfloat16
    fp8 = mybir.dt.float8e4
    AF = mybir.ActivationFunctionType

    nc = bacc.Bacc("TRN2", target_bir_lowering=False)

    edges_d = nc.dram_tensor("edges", [128, TOT], fp8, kind="ExternalInput")
    xt_d = nc.dram_tensor("xt", [F_X, SLOTS], bf16, kind="ExternalInput")
    dev_oh = cfg.get("dev_oh", False)
    if dev_oh:
        gid_d = nc.dram_tensor("gid", [1, SLOTS], bf16, kind="ExternalInput")
        iota_d = nc.dram_tensor("iota", [GR, 1], f32, kind="ExternalInput")
        if dev_oh is True:
            ones_d = nc.dram_tensor("ones", [1, GR], bf16, kind="ExternalInput")
    else:
        oh_d = nc.dram_tensor("oh", [GR, SLOTS], fp8, kind="ExternalInput")
    w1xu_d = nc.dram_tensor("w1xu", [128, H], bf16, kind="ExternalInput")
    w1e8d_d = nc.dram_tensor("w1e8d", [128, 2 * H], fp8, kind="ExternalInput")
    w1e8s_d = nc.dram_tensor("w1e8s", [128, H], fp8, kind="ExternalInput")
    w2_d = nc.dram_tensor("w2", [H, F_OUT], bf16, kind="ExternalInput")
    b1_d = nc.dram_tensor("b1", [H, 1], f32, kind="ExternalInput")
    out_d = nc.dram_tensor("outT", [F_OUT, NG, COLS], bf16, kind="ExternalOutput")

    with tile.TileContext(nc) as tc, ExitStack() as ctx:
        consts = ctx.enter_context(tc.tile_pool(name="consts", bufs=1))
        edge_pool = ctx.enter_context(
            tc.tile_pool(name="edges", bufs=cfg.get("edge_bufs", 3)))
        xoh_pool = ctx.enter_context(tc.tile_pool(name="xoh", bufs=3))
        hid_pool = ctx.enter_context(tc.tile_pool(
            name="hid", bufs=cfg.get("hid_bufs", 3)))
        out_pool = ctx.enter_context(tc.tile_pool(name="outs", bufs=3))
        psh_pool = ctx.enter_context(tc.tile_pool(
            name="psh", bufs=cfg.get("psh_bufs", 3), space="PSUM"))
        pso_pool = ctx.enter_context(tc.tile_pool(
            name="pso", bufs=cfg.get("pso_bufs", 3), space="PSUM"))

        w1xu_t = consts.tile([128, H], bf16)
        nc.sync.dma_start(w1xu_t[:], w1xu_d[:])
        w1e8d_t = consts.tile([128, 2 * H], fp8)
        nc.sync.dma_start(w1e8d_t[:], w1e8d_d[:])
        w1e8s_t = consts.tile([128, H], fp8)
        nc.sync.dma_start(w1e8s_t[:], w1e8s_d[:])
        w2_t = consts.tile([H, F_OUT], bf16)
        nc.sync.dma_start(w2_t[:], w2_d[:])
        b1_t = consts.tile([H, 1], f32)
        nc.sync.dma_start(b1_t[:], b1_d[:])
        if dev_oh:
            gid_t = consts.tile([1, SLOTS], bf16)
            nc.sync.dma_start(gid_t[:], gid_d[:])
            iota_t = consts.tile([GR, 1], f32)
            nc.sync.dma_start(iota_t[:], iota_d[:])
            if dev_oh is True:
                ones_t = consts.tile([1, GR], bf16)
                nc.sync.dma_start(ones_t[:], ones_d[:])
                psb_pool = ctx.enter_context(
                    tc.tile_pool(name="psb", bufs=2, space="PSUM"))
            else:
                gidb_pool = ctx.enter_context(
                    tc.tile_pool(name="gidb", bufs=3))

        OB = cfg.get("out_batch", 5)
        IB = cfg.get("in_batch", 5)
        EC = cfg.get("et_chunk", 5)  # groups per edge DMA
        assert NG % OB == 0 and NG % IB == 0 and NG % EC == 0
        DR = mybir.MatmulPerfMode.DoubleRow
        w1e8_v = w1e8d_t[:].rearrange("p (two h) -> p two h", two=2)
        comp_only = cfg.get("compute_only", False)

        outs = None
        et_ch = None
        et_ch_off = 0
        pend = []  # [(k_abs, hid tile), ...] awaiting W2
        DEFER = cfg.get("defer", 1)
        REPS = cfg.get("reps", 1)
        NCHUNK = NG // IB * REPS

        def issue_xoh(ck):
            """Prefetch the xoh tile for absolute chunk index ck."""
            k0 = (ck * IB) % NG
            xoh_t = xoh_pool.tile([128, IB * COLS], bf16)
            nc.gpsimd.dma_start(
                xoh_t[0:F_X, :], xt_d[:, k0 * COLS:(k0 + IB) * COLS])
            if not dev_oh:
                # SWDGE cast-DMA: fp8 one-hot expands to bf16 in flight
                nc.gpsimd.dma_start(
                    xoh_t[F_X:128, :], oh_d[:, k0 * COLS:(k0 + IB) * COLS])
            elif dev_oh == "bcast":
                gidb = gidb_pool.tile([GR, IB * COLS], bf16)
                nc.gpsimd.partition_broadcast(
                    gidb[:], gid_t[0:1, k0 * COLS:(k0 + IB) * COLS])
                nc.vector.tensor_scalar(
                    out=xoh_t[F_X:128, :],
                    in0=gidb[:], scalar1=iota_t[:, 0:1],
                    scalar2=None, op0=mybir.AluOpType.is_equal)
            else:
                for j in range(IB):
                    psb = psb_pool.tile([GR, COLS], f32)
                    nc.tensor.matmul(
                        psb[:], ones_t[:],
                        gid_t[:, (k0 + j) * COLS:(k0 + j + 1) * COLS],
                        start=True, stop=True)
                    nc.vector.tensor_scalar(
                        out=xoh_t[F_X:128, j * COLS:(j + 1) * COLS],
                        in0=psb[:], scalar1=iota_t[:, 0:1],
                        scalar2=None, op0=mybir.AluOpType.is_equal)
            return xoh_t

        def flush_w2(pk, phid):
            nonlocal outs
            k = pk % NG
            pso = pso_pool.tile([F_OUT, COLS], f32)
            nc.tensor.matmul(pso[:], w2_t[:], phid[:], start=True, stop=True)
            kb = k % OB
            if kb == 0:
                outs = out_pool.tile([F_OUT, OB * COLS], bf16)
            nc.vector.tensor_copy(outs[:, kb * COLS:(kb + 1) * COLS], pso[:])
            if kb == OB - 1:
                g0 = k - OB + 1
                # emitted AFTER the next chunk's prefetch DMAs, so this
                # (data-dependent) DMA never blocks prefetch in the FIFO
                out_eng = {"act": nc.scalar, "sync": nc.sync,
                           "pool": nc.gpsimd}[cfg.get("out_q", "pool")]
                out_eng.dma_start(
                    out_d[:, g0:k + 1, :],
                    outs[:].rearrange("f (g c) -> f g c", g=OB))

        PRE = cfg.get("xoh_pre", 1)
        xoh_q = [issue_xoh(p) for p in range(PRE)]
        xoh_t = None
        for k_r in range(NG * REPS):
            k = k_r % NG
            NP = NPS[k]
            o = int(off[k])
            if k % EC == 0:
                csz = int(sum(NPS[k:k + EC])) * COLS
                et_ch = edge_pool.tile([128, csz], fp8)
                pat = cfg.get("et_pat")
                if pat:
                    et_eng = {"S": nc.sync, "A": nc.scalar,
                              "P": nc.gpsimd}[pat[(k_r // EC) % len(pat)]]
                elif cfg.get("et_q3"):
                    et_eng = [nc.sync, nc.scalar, nc.gpsimd][(k_r // EC) % 3]
                elif cfg.get("et_split"):
                    et_eng = [nc.sync, nc.scalar][(k_r // EC) % 2]
                else:
                    et_eng = nc.sync
                if comp_only:
                    et_eng.dma_start(et_ch[:, 0:4], edges_d[:, o:o + 4])
                else:
                    et_eng.dma_start(et_ch[:], edges_d[:, o:o + csz])
                et_ch_off = o
            et = et_ch
            eo = o - et_ch_off
            ki = k % IB
            if ki == 0:
                ck = k_r // IB
                xoh_t = xoh_q.pop(0)
                if ck + PRE < NCHUNK:
                    xoh_q.append(issue_xoh(ck + PRE))

            psh = psh_pool.tile([H, COLS], f32)
            for j in range(NP // 2):
                rhs = et[:, eo + 2 * j * COLS:eo + (2 * j + 2) * COLS].rearrange(
                    "p (two c) -> p two c", two=2)
                nc.tensor.matmul(
                    psh[:], w1e8_v, rhs,
                    start=(j == 0), stop=False, perf_mode=DR,
                )
            if NP % 2:
                nc.tensor.matmul(
                    psh[:], w1e8s_t[:], et[:, eo + (NP - 1) * COLS:eo + NP * COLS],
                    start=(NP == 1), stop=False,
                )
            nc.tensor.matmul(
                psh[:], w1xu_t[:], xoh_t[:, ki * COLS:(ki + 1) * COLS],
                start=False, stop=True)

            hid = hid_pool.tile([H, COLS], bf16)
            if cfg.get("relu_eng", "act") == "dve":
                nc.vector.tensor_scalar(
                    out=hid[:], in0=psh[:], scalar1=b1_t[:], scalar2=0.0,
                    op0=mybir.AluOpType.add, op1=mybir.AluOpType.max)
            else:
                nc.scalar.activation(hid[:], psh[:], AF.Relu, bias=b1_t[:],
                                     scale=1.0)

            pend.append((k_r, hid))
            if len(pend) > DEFER:
                flush_w2(*pend.pop(0))
        for p in pend:
            flush_w2(*p)

    nc.finalize()
    return nc


def _get_program(cfg):
    key = tuple(sorted((k, v) for k, v in cfg.items()))
    if key not in _CACHE:
        _CACHE[key] = _build(cfg)
    return _CACHE[key]


def run(inputs, cfg=None, trace=False):
    from concourse.bass_utils import run_bass_kernel_spmd

    cfg = dict(CFG if cfg is None else cfg)
    in_maps, meta = _preprocess(inputs, cfg)
    nc = _get_program(cfg)
    res = run_bass_kernel_spmd(
        nc, in_maps, list(range(cfg["n_cores"])), trace=trace)
    out = _postprocess(res.results, meta, cfg)
    return out, res


def kernel(**inputs):
    return run(inputs)[0]
